# revision 13
# baseline (speedup 1.0000x reference)
"""Trainium2 Bass kernel for nn_CrossAttentionBlock (GroupNorm + 1x1-conv Q +
cross-attention over cond + output projection + residual).

Full-input contract: kernel(**inputs) takes the complete unsharded inputs and
returns the full [16, 512, 64, 64] float32 output.  Internally shards
data-parallel over batch across 8 NeuronCores (2 batches per core), runs one
SPMD Bass/Tile kernel via run_bass_kernel_spmd, and concatenates the results.

Layout strategy (per core, per batch, channels-first [C, HW] everywhere):
  x streamed in f32 per 128-channel tile (SP DMA ring), groupnorm stats via
  DVE bn_stats on the f32 staging tile, then cast to a bf16-resident copy
  (Pool tensor_copy) used by the GN apply and the residual add -- bf16
  residency halves SBUF so x double-buffers across the 2 batches.
  Weights/cond/colv load on the ACT DMA ring in parallel; output stores go
  out on the Pool SWDGE ring.  Per hw-chunk (512 cols):
    xn = GpSimd tensor_scalar(x_bf*sc + tc) in bf16
    q  = qwT.T @ xn; PSUM->SBUF cast + q_b bias on ACT  [C, 512]  PE
    per head h (hd=64): logits^T = kT_h.T @ q_h  [77, 512] PE (head pairs at
      partition bases 0/64 -> concurrent PE row-groups)
    exp via ACT -> per-head [77, 512] bf16 (no max subtraction; logits O(10))
    sums_h = ones77.T @ exp_h replicated over the head's 64 out channels (PE)
    rcp = DVE reciprocal_approx_fast (NOT nc.vector.reciprocal: that one is a
      bit-exact iterative divide at ~6 cycles/elem on HW)
    av = v_h @ exp_h [64, 512] PE pair-packed; normalize = DVE tensor_mul
    out = pwT.T @ av_norm; residual + proj bias fused in one DVE
      scalar_tensor_tensor: osb = (psum + pb_col) + x_bf
    one coalesced DMA per chunk stores [128, 4, 512] -> out[b, :, cs]
Weights transposed/cast to bf16 and small constants packed on the host.
"""

import sys

for _p in ("/opt/trn_rl_repo",):
    if _p not in sys.path:
        sys.path.append(_p)

from contextlib import ExitStack

import numpy as np
import ml_dtypes

import concourse.bacc as bacc
import concourse.tile as tile
from concourse import mybir
from concourse.bass_utils import run_bass_kernel_spmd

BF16 = ml_dtypes.bfloat16

N_CORES = 8
B, C, H, W = 16, 512, 64, 64
HW = H * W                      # 4096
L, CD = 77, 768
NH, HD = 8, 64                  # heads, head dim
NG, GS = 32, 16                 # groups, channels per group
EPS = 1e-6
B_LOC = B // N_CORES            # 2
NT = C // 128                   # 4 channel tiles
KT = CD // 128                  # 6 cond-dim tiles
CH = 512                        # hw chunk
NCH = HW // CH                  # 8
GPT = 128 // GS                 # 8 groups per 128-channel tile

FP8_OUT = True                  # fp8e4 DoubleRow output projection
FP8_Q = False                   # fp8e4 DoubleRow q projection
W8S = 64.0                      # fp8 weight pre-scale (w ~ N(0, 0.02^2))
FP8 = ml_dtypes.float8_e4m3


def _build_nc(nch=NCH, reps=1):
    f32 = mybir.dt.float32
    bf16 = mybir.dt.bfloat16
    f8 = mybir.dt.float8e4
    DR = mybir.MatmulPerfMode.DoubleRow
    nc = bacc.Bacc("TRN2", target_bir_lowering=False, debug=False)

    x_d = nc.dram_tensor("x", [B_LOC, C, HW], f32, kind="ExternalInput").ap()
    condT_d = nc.dram_tensor("condT", [B_LOC, CD, L], bf16, kind="ExternalInput").ap()
    qwT_d = nc.dram_tensor("qwT", [C, C], f8 if FP8_Q else bf16,
                           kind="ExternalInput").ap()
    kwT_d = nc.dram_tensor("kwT", [CD, C], bf16, kind="ExternalInput").ap()
    vwT_d = nc.dram_tensor("vwT", [CD, C], bf16, kind="ExternalInput").ap()
    pwT_d = nc.dram_tensor("pwT", [C, C], f8 if FP8_OUT else bf16,
                           kind="ExternalInput").ap()
    colv_d = nc.dram_tensor("colv", [128, 20], f32, kind="ExternalInput").ap()
    scl_d = nc.dram_tensor("scale128", [128, 1], f32, kind="ExternalInput").ap()
    vbbc_d = nc.dram_tensor("vbbc", [L, C], f32, kind="ExternalInput").ap()
    g16_d = nc.dram_tensor("g16", [128, GPT], f32, kind="ExternalInput").ap()
    g16T_d = nc.dram_tensor("g16T", [GPT, 128], f32, kind="ExternalInput").ap()
    out_d = nc.dram_tensor("out", [B_LOC, C, HW], f32, kind="ExternalOutput").ap()

    AO = mybir.AluOpType
    AF = mybir.ActivationFunctionType

    with tile.TileContext(nc) as tc, ExitStack() as ctx:
        # --- pools ---
        wp = ctx.enter_context(tc.tile_pool(name="weights", bufs=1))
        sb1 = ctx.enter_context(tc.tile_pool(name="work1", bufs=2))
        sb2 = ctx.enter_context(tc.tile_pool(name="work2", bufs=2))
        sb3 = ctx.enter_context(tc.tile_pool(name="work3", bufs=2))
        # PSUM: q(2) + at(2, shared qk/av) + sums(2) + o(2) = 8 banks
        ps_q = ctx.enter_context(tc.tile_pool(name="ps_q", bufs=2, space="PSUM"))
        ps_at = ctx.enter_context(tc.tile_pool(name="ps_at", bufs=2, space="PSUM"))
        ps_sm = ctx.enter_context(tc.tile_pool(name="ps_sm", bufs=2, space="PSUM"))
        ps_o = ctx.enter_context(tc.tile_pool(name="ps_o", bufs=2, space="PSUM"))

        # --- persistent weights/constants: one DMA each, on the ACT ring ---
        qwall = wp.tile([128, NT, C], f8 if FP8_Q else bf16, tag="qwall")
        pwall = wp.tile([128, NT, C], f8 if FP8_OUT else bf16, tag="pwall")
        kwall = wp.tile([128, KT, C], bf16, tag="kwall")
        vwall = wp.tile([128, KT, C], bf16, tag="vwall")
        nc.scalar.dma_start(qwall[:], qwT_d.rearrange("(j p) c -> p j c", p=128))
        nc.scalar.dma_start(pwall[:], pwT_d.rearrange("(j p) c -> p j c", p=128))
        nc.scalar.dma_start(kwall[:], kwT_d.rearrange("(j p) c -> p j c", p=128))
        nc.scalar.dma_start(vwall[:], vwT_d.rearrange("(j p) c -> p j c", p=128))
        qwT = [qwall[:, j, :] for j in range(NT)]
        pwT = [pwall[:, j, :] for j in range(NT)]
        kwT = [kwall[:, j, :] for j in range(KT)]
        vwT = [vwall[:, j, :] for j in range(KT)]

        g16 = wp.tile([128, GPT], f32, tag="g16")
        nc.scalar.dma_start(g16[:], g16_d[:, :])
        g16T = wp.tile([GPT, 128], f32, tag="g16T")
        nc.scalar.dma_start(g16T[:], g16T_d[:, :])
        # columns: 0-3 gamma, 4-7 beta, 8-11 qb, 12-15 kb*scale, 16-19 pb
        colv = wp.tile([128, 20], f32, tag="colv")
        nc.scalar.dma_start(colv[:], colv_d[:, :])
        scale_col = wp.tile([128, 1], f32, tag="scale_col")
        nc.scalar.dma_start(scale_col[:], scl_d[:, :])
        vb_bc = wp.tile([L, C], f32, tag="vb_bc")
        nc.scalar.dma_start(vb_bc[:], vbbc_d[:, :])
        ones77 = wp.tile([L, 64], bf16, tag="ones77")
        nc.gpsimd.memset(ones77[:], 1.0)
        eps_col = wp.tile([GPT, 1], f32, tag="eps_col")
        nc.gpsimd.memset(eps_col[:], EPS)

        rep_ctx = tc.For_i(0, reps, 1) if reps > 1 else None
        if rep_ctx is not None:
            rep_ctx.__enter__()
        for b in range(B_LOC):
            # ---------- stream x per half-tile on both DMA rings; stats on the
            # f32 staging tile; keep a bf16 copy (+proj bias folded in) ------
            xball = sb1.tile([128, NT, HW], bf16, tag="x", name="x")
            bnout = sb2.tile([128, NT, 8, 6], f32, tag="bnout")
            for t in range(NT):
                xst = sb1.tile([128, HW], f32, tag="xst", name=f"xst{t}")
                eng = nc.sync if t % 2 == 0 else nc.scalar
                hh = HW // 2
                for h2 in range(2):
                    hs = slice(hh * h2, hh * (h2 + 1))
                    eng.dma_start(xst[:, hs],
                                  x_d[b, 128 * t:128 * (t + 1), hs])
                    for j in range(NCH // 2):
                        cj = (NCH // 2) * h2 + j
                        nc.vector.bn_stats(bnout[:, t, cj, :],
                                           xst[:, CH * cj:CH * (cj + 1)])
                    # bf16 resident copy on ACT; fold proj bias into x so
                    # the residual op has a free slot for the fp8 descale
                    nc.scalar.activation(xball[:, t, hs], xst[:, hs],
                                         AF.Identity,
                                         bias=colv[:, 16 + t:17 + t])
            # per-channel mean/var -> gin cols 2t = mean, 2t+1 = E[x^2]
            gin = sb2.tile([128, 2 * NT], f32, tag="gin")
            tmp4 = sb2.tile([128, NT], f32, tag="tmp4")
            for t in range(NT):
                nc.vector.bn_aggr(gin[:, 2 * t:2 * t + 2], bnout[:, t, :, :])
            nc.vector.tensor_mul(tmp4[:], gin[:, 0:2 * NT:2], gin[:, 0:2 * NT:2])
            nc.vector.tensor_add(gin[:, 1:2 * NT:2], gin[:, 1:2 * NT:2], tmp4[:])
            # group reduce (sum over each 16-channel group) in one matmul
            gst = ps_sm.tile([GPT, 2 * NT], f32, tag="sm")
            nc.tensor.matmul(gst[:], g16[:], gin[:], start=True, stop=True)
            # per-group mu / rsig
            gw = sb2.tile([GPT, 2 * NT], f32, tag="gw")
            tmpg = sb2.tile([GPT, NT], f32, tag="tmpg")
            nc.vector.tensor_scalar_mul(gw[:], gst[:], 1.0 / GS)
            nc.vector.tensor_mul(tmpg[:], gw[:, 0:2 * NT:2], gw[:, 0:2 * NT:2])
            nc.vector.tensor_sub(gw[:, 1:2 * NT:2], gw[:, 1:2 * NT:2], tmpg[:])
            tmpg2 = sb2.tile([GPT, NT], f32, tag="tmpg2")
            nc.scalar.activation(tmpg2[:], gw[:, 1:2 * NT:2], AF.Sqrt,
                                 bias=eps_col[:])
            nc.vector.reciprocal(gw[:, 1:2 * NT:2], tmpg2[:])
            # scatter groups -> channels in one matmul: cols 2t=mu, 2t+1=rsig
            cst = ps_sm.tile([128, 2 * NT], f32, tag="sm")
            nc.tensor.matmul(cst[:], g16T[:], gw[:], start=True, stop=True)
            # per-channel scale/bias: sc = gamma*rsig, tc = beta - mu*sc
            scb = sb2.tile([128, 2 * NT], f32, tag="scb")
            tmpc = sb2.tile([128, NT], f32, tag="tmpc")
            nc.vector.tensor_mul(scb[:, 0:2 * NT:2], colv[:, 0:NT],
                                 cst[:, 1:2 * NT:2])
            nc.vector.tensor_mul(tmpc[:], cst[:, 0:2 * NT:2],
                                 scb[:, 0:2 * NT:2])
            nc.vector.tensor_sub(scb[:, 1:2 * NT:2], colv[:, NT:2 * NT],
                                 tmpc[:])
            # xball carries x + pb, so shift the GN bias: tc -= sc*pb
            nc.vector.tensor_mul(tmpc[:], scb[:, 0:2 * NT:2],
                                 colv[:, 16:20])
            nc.vector.tensor_sub(scb[:, 1:2 * NT:2], scb[:, 1:2 * NT:2],
                                 tmpc[:])

            # ---------- K^T and V projections from cond ----------
            cTall = sb2.tile([128, KT, L], bf16, tag="cT")
            nc.scalar.dma_start(cTall[:],
                                condT_d[b].rearrange("(j p) l -> p j l", p=128))
            kT = [sb2.tile([128, L], bf16, tag=f"kT{t}", name=f"kT{t}")
                  for t in range(NT)]
            v_sb = sb2.tile([L, C], bf16, tag="v_sb")
            for t in range(NT):
                cs = slice(128 * t, 128 * (t + 1))
                pk = ps_q.tile([128, CH], f32, tag="q")
                for j in range(KT):
                    nc.tensor.matmul(pk[:, 0:L], kwT[j][:, cs], cTall[:, j, :],
                                     start=(j == 0), stop=(j == KT - 1))
                nc.scalar.activation(kT[t][:], pk[:, 0:L], AF.Identity,
                                     bias=colv[:, 12 + t:13 + t],
                                     scale=scale_col[:])
                pv = ps_at.tile([128, CH], f32, tag="at")
                for j in range(KT):
                    nc.tensor.matmul(pv[0:L, 0:128], cTall[:, j, :],
                                     vwT[j][:, cs],
                                     start=(j == 0), stop=(j == KT - 1))
                nc.vector.tensor_add(v_sb[:, cs], pv[0:L, 0:128], vb_bc[:, cs])

            # ---------- hw-chunk pipeline ----------
            for cix in range(nch):
                cs = slice(CH * cix, CH * (cix + 1))
                # groupnorm apply on GpSimd (bf16/fp8 out)
                xna = sb2.tile([128, NT, CH], f8 if FP8_Q else bf16, tag="xn",
                               name="xn")
                for t in range(NT):
                    nc.gpsimd.tensor_scalar(xna[:, t, :], xball[:, t, cs],
                                            scb[:, 2 * t:2 * t + 1],
                                            scb[:, 2 * t + 1:2 * t + 2],
                                            op0=AO.mult, op1=AO.add)
                # q projection; bias fused into the ACT PSUM->SBUF cast
                q_sb = [sb2.tile([128, CH], bf16, tag=f"q{m}", name=f"qsb{m}")
                        for m in range(NT)]
                for m in range(NT):
                    ms = slice(128 * m, 128 * (m + 1))
                    pq = ps_q.tile([128, CH], f32, tag="q")
                    if FP8_Q:
                        for j in range(NT // 2):
                            nc.tensor.matmul(pq[:],
                                             qwall[:, 2 * j:2 * j + 2, ms],
                                             xna[:, 2 * j:2 * j + 2, :],
                                             start=(j == 0),
                                             stop=(j == NT // 2 - 1),
                                             perf_mode=DR)
                        nc.scalar.activation(q_sb[m][:], pq[:], AF.Identity,
                                             bias=colv[:, 8 + m:9 + m],
                                             scale=1.0 / W8S)
                    else:
                        for k in range(NT):
                            nc.tensor.matmul(pq[:], qwT[k][:, ms],
                                             xna[:, k, :],
                                             start=(k == 0),
                                             stop=(k == NT - 1))
                        nc.scalar.activation(q_sb[m][:], pq[:], AF.Identity,
                                             bias=colv[:, 8 + m:9 + m])
                # attention: per-head logits^T -> exp (bf16)
                eh = [sb2.tile([L, CH], bf16, tag=f"eh{h}", name=f"eh{h}")
                      for h in range(NH)]
                for h in range(NH):
                    t_, off = h // 2, 64 * (h % 2)
                    pqk = ps_at.tile([128, CH], f32, tag="at")
                    nc.tensor.matmul(pqk[0:L, :], kT[t_][off:off + 64, :],
                                     q_sb[t_][off:off + 64, :],
                                     start=True, stop=True)
                    nc.scalar.activation(eh[h][:], pqk[0:L, :], AF.Exp)
                # AV (pair-packed) + PE-replicated sums + normalize
                prja = sb2.tile([128, NT, CH], f8 if FP8_OUT else bf16,
                                tag="pi", name="pi")
                for p in range(NT):
                    psm = ps_sm.tile([128, CH], f32, tag="sm")
                    pav = ps_at.tile([128, CH], f32, tag="at")
                    for h in (2 * p, 2 * p + 1):
                        off = 64 * (h % 2)
                        # sum of exp replicated over this head's 64 rows
                        nc.tensor.matmul(psm[off:off + 64, :], ones77[:],
                                         eh[h][:], start=True, stop=True)
                        nc.tensor.matmul(pav[off:off + 64, :],
                                         v_sb[:, 64 * h:64 * h + 64], eh[h][:],
                                         start=True, stop=True)
                    rcp = sb2.tile([128, CH], f32, tag=f"rcp{p % 2}",
                                   name=f"rcp{p}")
                    nc.vector.reciprocal_approx_fast(rcp[:], psm[:])
                    nc.vector.tensor_mul(prja[:, p, :], pav[:], rcp[:])
                # output projection; residual fused in one DVE op (x carries
                # pb already); one coalesced store per chunk on the Pool ring
                osb = sb3.tile([128, NT, CH], f32, tag="osb", name="osb")
                for m in range(NT):
                    ms = slice(128 * m, 128 * (m + 1))
                    po = ps_o.tile([128, CH], f32, tag="o")
                    if FP8_OUT:
                        for j in range(NT // 2):
                            nc.tensor.matmul(po[:],
                                             pwall[:, 2 * j:2 * j + 2, ms],
                                             prja[:, 2 * j:2 * j + 2, :],
                                             start=(j == 0),
                                             stop=(j == NT // 2 - 1),
                                             perf_mode=DR)
                        nc.vector.scalar_tensor_tensor(
                            osb[:, m, :], po[:], 1.0 / W8S,
                            xball[:, m, cs], op0=AO.mult, op1=AO.add)
                    else:
                        for k in range(NT):
                            nc.tensor.matmul(po[:], pwT[k][:, ms],
                                             prja[:, k, :],
                                             start=(k == 0),
                                             stop=(k == NT - 1))
                        nc.vector.tensor_add(osb[:, m, :], po[:],
                                             xball[:, m, cs])
                out_view = out_d[b, :, cs].rearrange("(m p) w -> p m w", p=128)
                nc.sync.dma_start(out_view, osb[:])
        if rep_ctx is not None:
            rep_ctx.__exit__(None, None, None)

    nc.compile()
    return nc


_NC_CACHE = None


def _get_nc():
    global _NC_CACHE
    if _NC_CACHE is None:
        _NC_CACHE = _build_nc()
    return _NC_CACHE


def make_in_maps(x, cond, gamma, beta, q_w, q_b, k_w, k_b, v_w, v_b,
                 proj_w, proj_b, scale):
    x = np.asarray(x, np.float32).reshape(B, C, HW)
    condT = np.asarray(cond, np.float32).transpose(0, 2, 1).astype(BF16)
    qwTf = np.ascontiguousarray(np.asarray(q_w, np.float32).T)
    qwT = (qwTf * W8S).astype(FP8) if FP8_Q else qwTf.astype(BF16)
    kwT = np.ascontiguousarray(np.asarray(k_w, np.float32).T).astype(BF16)
    vwT = np.ascontiguousarray(np.asarray(v_w, np.float32).T).astype(BF16)
    pwTf = np.ascontiguousarray(np.asarray(proj_w, np.float32).T)
    pwT = (pwTf * W8S).astype(FP8) if FP8_OUT else pwTf.astype(BF16)
    g16 = np.zeros((128, GPT), np.float32)
    for p in range(128):
        g16[p, p // GS] = 1
    g16T = np.ascontiguousarray(g16.T)
    sc = float(np.asarray(scale).reshape(-1)[0])
    colv = np.zeros((128, 20), np.float32)
    for t in range(NT):
        s = slice(128 * t, 128 * (t + 1))
        colv[:, t] = np.asarray(gamma, np.float32)[s]
        colv[:, 4 + t] = np.asarray(beta, np.float32)[s]
        colv[:, 8 + t] = np.asarray(q_b, np.float32)[s]
        colv[:, 12 + t] = np.asarray(k_b, np.float32)[s] * sc
        colv[:, 16 + t] = np.asarray(proj_b, np.float32)[s]
    vbbc = np.broadcast_to(np.asarray(v_b, np.float32)[None, :], (L, C))
    com = dict(
        qwT=qwT, kwT=kwT, vwT=vwT, pwT=pwT,
        colv=colv,
        scale128=np.full((128, 1), sc, np.float32),
        vbbc=np.ascontiguousarray(vbbc),
        g16=g16, g16T=g16T,
    )
    in_maps = []
    for cix in range(N_CORES):
        bs = slice(B_LOC * cix, B_LOC * (cix + 1))
        m = dict(com)
        m["x"] = np.ascontiguousarray(x[bs])
        m["condT"] = np.ascontiguousarray(condT[bs])
        in_maps.append(m)
    return in_maps


def kernel(x, cond, gamma, beta, q_w, q_b, k_w, k_b, v_w, v_b,
           proj_w, proj_b, scale):
    nc = _get_nc()
    in_maps = make_in_maps(x, cond, gamma, beta, q_w, q_b, k_w, k_b,
                           v_w, v_b, proj_w, proj_b, scale)
    res = run_bass_kernel_spmd(nc, in_maps, core_ids=list(range(N_CORES)))
    out = np.concatenate([r["out"] for r in res.results], axis=0)
    return out.reshape(B, C, H, W).astype(np.float32)
